# revision 1
# baseline (speedup 1.0000x reference)
"""Trainium2 Bass kernel for nn_MultiHeadAttention (N=2048, D=1024, H=16, causal).

Sharding: the 16 heads are split across the 8 NeuronCores (2 heads/core,
tensor-parallel on the head dim, per the sharding hint).  Each core:
  - projects Q^T/K^T (its 128 head-dims x full sequence) and V for its heads,
  - computes causal attention in scores-TRANSPOSED layout ([nk, nq] blocks):
    softmax runs along the nk partition axis with no max-subtraction (scores
    are O(1) here so exp is safe), and the softmax denominator falls out of
    the PV matmul via a ones-column appended to V,
  - applies the 128-column slice of Wo, giving a partial [2048, 1024] output.
The host sums the 8 partial outputs and adds bo (the "all-reduce after W_o"
step; cheaper done host-side than an on-device AllReduce of 8.4 MB/core).

Structure: "t-outer" — for each of the 4 nq column tiles, input column tiles
are DMA-streamed and projected, both heads' attention for that nq range runs
over nk blocks 0..4t+3 (causally trimmed), and normalization + output
projection + output DMA for those 4 row-blocks happen immediately.  This
overlaps input DMA, PE matmuls, ScalarE softmax, and output DMA across the
whole kernel (modeled ~133 us/core vs ~210 us for a phase-serial version).

Dtypes: float32r (TF32-like PE mode, full rate at free-dim >= 256) for all
matmul operands, fp32 PSUM accumulation and softmax.  Measured end-to-end
relative error vs the fp32 jax reference: ~2e-4.
"""
import os
import sys

for _p in ("/opt/trn_rl_repo", "/root/.axon_site/_ro/trn_rl_repo"):
    if os.path.isdir(_p) and _p not in sys.path:
        sys.path.append(_p)

import numpy as np

import concourse.bass as bass
import concourse.mybir as mybir
from concourse import bacc
from concourse.bass_utils import run_bass_kernel_spmd
from concourse.tile import TileContext
from contextlib import ExitStack

N = 2048
D = 1024
NCORES = 8
DL = 128

F32 = mybir.dt.float32
F32R = mybir.dt.float32r
BF16 = mybir.dt.bfloat16

DT = F32R


def build_nc(opts=None):
    o = dict(qk_dt=F32R, v_dt=F32R, pdt=F32R)
    if opts:
        o.update(opts)
    qk_dt = o["qk_dt"]
    v_dt = o["v_dt"]
    pdt = o["pdt"]
    nc = bacc.Bacc("TRN2", target_bir_lowering=False, debug=False,
                   num_devices=NCORES)

    qT = nc.dram_tensor("qT", [D, N], qk_dt, kind="ExternalInput")
    kT = nc.dram_tensor("kT", [D, N], qk_dt, kind="ExternalInput")
    vT = nc.dram_tensor("vT", [D, N], v_dt, kind="ExternalInput")
    wqT = nc.dram_tensor("wqT", [D, DL], qk_dt, kind="ExternalInput")
    wkT = nc.dram_tensor("wkT", [D, DL], qk_dt, kind="ExternalInput")
    wvT = nc.dram_tensor("wvT", [D, DL], v_dt, kind="ExternalInput")
    bqkv = nc.dram_tensor("bqkv", [DL, 3], F32, kind="ExternalInput")
    bvrow = nc.dram_tensor("bvrow", [1, DL], v_dt, kind="ExternalInput")
    woT = nc.dram_tensor("woT", [DL, D], DT, kind="ExternalInput")
    out = nc.dram_tensor("out", [N, D], F32, kind="ExternalOutput")

    AF = mybir.ActivationFunctionType

    with TileContext(nc) as tc, ExitStack() as ctx:
        const = ctx.enter_context(tc.tile_pool(name="const", bufs=1))
        big = ctx.enter_context(tc.tile_pool(name="big", bufs=1))
        stream = ctx.enter_context(tc.tile_pool(name="stream", bufs=12))
        vstream = ctx.enter_context(tc.tile_pool(name="vstream", bufs=10))
        probs_pool = ctx.enter_context(tc.tile_pool(name="probs", bufs=4))
        recip_pool = ctx.enter_context(tc.tile_pool(name="recip", bufs=2))
        outp = ctx.enter_context(tc.tile_pool(name="outp", bufs=4))

        # constants (scalar queue, before any activation work exists)
        wq = const.tile([128, 8, DL], qk_dt)
        nc.scalar.dma_start(wq[:], wqT.rearrange("(j p) d -> p j d", p=128))
        wk = const.tile([128, 8, DL], qk_dt)
        nc.scalar.dma_start(wk[:], wkT.rearrange("(j p) d -> p j d", p=128))
        wv = const.tile([128, 8, DL], v_dt)
        nc.scalar.dma_start(wv[:], wvT.rearrange("(j p) d -> p j d", p=128))
        wo = const.tile([128, D], DT)
        nc.scalar.dma_start(wo[:], woT[:])
        bias_cols = const.tile([128, 3], F32)
        nc.scalar.dma_start(bias_cols[:], bqkv[:])
        if o["v_dt"] != F32R:
            bv_row = const.tile([1, DL], v_dt)
            nc.scalar.dma_start(bv_row[:], bvrow[:])
            ones_n = const.tile([1, 128], v_dt)
            nc.vector.memset(ones_n[:], 1.0)
        ones64 = const.tile([1, 64], F32)
        nc.vector.memset(ones64[:], 1.0)
        if o["v_dt"] == F32R:
            from concourse.masks import make_identity
            ident = const.tile([128, 128], F32)
            make_identity(nc, ident[:])

        QTs = [big.tile([128, 512], DT, name=f"QT{t}") for t in range(4)]
        KTs = [big.tile([128, 512], DT, name=f"KT{t}") for t in range(4)]
        Vaug0 = big.tile([128, 16, 65], pdt)
        Vaug1 = big.tile([128, 16, 65], pdt)
        if pdt == F32R:
            onescol = const.tile([128, 16, 1], F32)
            nc.vector.memset(onescol[:], 1.0)
            nc.vector.tensor_copy(Vaug0[:, :, 64:65], onescol[:])
            nc.vector.tensor_copy(Vaug1[:, :, 64:65], onescol[:])
        else:
            nc.vector.memset(Vaug0[:, :, 64:65], 1.0)
            nc.vector.memset(Vaug1[:, :, 64:65], 1.0)
        attnT_n = big.tile([128, N], DT)
        denom0 = big.tile([1, N], F32)
        denom1 = big.tile([1, N], F32)

        with tc.tile_pool(name="sc_ps", bufs=3, space="PSUM") as sc_ps, \
             tc.tile_pool(name="pv_ps", bufs=1, space="PSUM") as pv_ps, \
             tc.tile_pool(name="proj_ps", bufs=1, space="PSUM") as proj_ps, \
             tc.tile_pool(name="wo_ps", bufs=2, space="PSUM") as wo_ps:

            for t in range(4):
                # ---- Q/K column-tile projections ----
                for src, w, bcol, dst in ((qT, wq, 0, QTs[t]),
                                          (kT, wk, 1, KTs[t])):
                    ps = proj_ps.tile([128, 512], F32, name="proj")
                    for j in range(8):
                        xt = stream.tile([128, 512], qk_dt, name="xc")
                        eng = (nc.sync, nc.scalar)[j % 2]
                        eng.dma_start(
                            xt[:],
                            src[128 * j:128 * (j + 1), 512 * t:512 * (t + 1)])
                        nc.tensor.matmul(ps[:], w[:, j, :], xt[:],
                                         start=(j == 0), stop=(j == 7))
                    nc.vector.tensor_scalar_add(dst[:], ps[:],
                                                bias_cols[:, bcol:bcol + 1])
                # ---- V blocks 4t..4t+3 (layout [n, dk], heads split) ----
                vgc = []
                for j in range(8):
                    vc = vstream.tile([128, 512], v_dt, name="vc")
                    (nc.scalar if j % 2 else nc.sync).dma_start(
                        vc[:], vT[128 * j:128 * (j + 1), 512 * t:512 * (t + 1)])
                    vgc.append(vc)
                if o["v_dt"] == F32R:
                    # VT column tile then PE-transpose into Vaug
                    ps = proj_ps.tile([128, 512], F32, name="proj")
                    for j in range(8):
                        nc.tensor.matmul(ps[:], wv[:, j, :], vgc[j][:],
                                         start=(j == 0), stop=(j == 7))
                    vtt = vstream.tile([128, 512], F32, name="vtt")
                    nc.vector.tensor_scalar_add(vtt[:], ps[:],
                                                bias_cols[:, 2:3])
                    for bb in range(4):
                        b = 4 * t + bb
                        tp = proj_ps.tile([128, 512], F32, name="proj")
                        nc.tensor.transpose(tp[:, 0:128],
                                            vtt[:, 128 * bb:128 * (bb + 1)],
                                            ident[:])
                        nc.vector.tensor_copy(Vaug0[:, b, 0:64], tp[:, 0:64])
                        nc.vector.tensor_copy(Vaug1[:, b, 0:64], tp[:, 64:128])
                else:
                    for bb in range(4):
                        b = 4 * t + bb
                        ps = proj_ps.tile([128, 512], F32, name="proj")
                        for j in range(8):
                            nc.tensor.matmul(ps[:, 0:128],
                                             vgc[j][:, 128 * bb:128 * (bb + 1)],
                                             wv[:, j, :],
                                             start=(j == 0), stop=False)
                        nc.tensor.matmul(ps[:, 0:128], ones_n[:], bv_row[:],
                                         start=False, stop=True)
                        nc.vector.tensor_copy(Vaug0[:, b, 0:64], ps[:, 0:64])
                        nc.vector.tensor_copy(Vaug1[:, b, 0:64], ps[:, 64:128])

                # ---- attention for nq tile t, both heads ----
                for h in range(2):
                    Vaug = (Vaug0, Vaug1)[h]
                    denom = (denom0, denom1)[h]
                    pvh = pv_ps.tile([65, 512], F32, name=f"pvh{h}")
                    prev = None
                    for b in range(4 * t + 4):
                        sc = sc_ps.tile([128, 512], F32, name="sc")
                        nc.tensor.matmul(
                            sc[:],
                            KTs[b // 4][64 * h:64 * (h + 1),
                                        128 * (b % 4):128 * (b % 4 + 1)],
                            QTs[t][64 * h:64 * (h + 1), :],
                            start=True, stop=True, tile_position=(64 * h, 0))
                        probs = probs_pool.tile([128, 512], pdt, name="probs")
                        nc.scalar.activation(probs[:], sc[:], AF.Exp,
                                             scale=0.125)
                        if b >= 4 * t:
                            off = 128 * (b - 4 * t)
                            nc.gpsimd.affine_select(
                                out=probs[:, 0:off + 128],
                                in_=probs[:, 0:off + 128],
                                compare_op=mybir.AluOpType.is_ge, fill=0.0,
                                base=-off, pattern=[[1, off + 128]],
                                channel_multiplier=-1)
                        if prev is not None:
                            pb, pp = prev
                            nc.tensor.matmul(pvh[:], Vaug[:, pb, :], pp[:],
                                             start=(pb == 0),
                                             stop=(pb == 4 * t + 3))
                        prev = (b, probs)
                    pb, pp = prev
                    nc.tensor.matmul(pvh[:], Vaug[:, pb, :], pp[:],
                                     start=(pb == 0), stop=(pb == 4 * t + 3))
                    # finalize softmax for this head / column tile
                    nc.vector.tensor_copy(denom[:, 512 * t:512 * (t + 1)],
                                          pvh[64:65, :])
                    bc = sc_ps.tile([64, 512], F32, name="sc")
                    nc.tensor.matmul(bc[:], ones64[:],
                                     denom[:, 512 * t:512 * (t + 1)],
                                     start=True, stop=True)
                    rc = recip_pool.tile([64, 512], F32, name="rc")
                    nc.vector.reciprocal(rc[:], bc[:])
                    nc.vector.tensor_mul(
                        attnT_n[64 * h:64 * (h + 1), 512 * t:512 * (t + 1)],
                        pvh[0:64, :], rc[:])

                # ---- output projection for row blocks 4t..4t+3 ----
                for m in range(4 * t, 4 * t + 4):
                    for u in range(2):
                        wps = wo_ps.tile([128, 512], F32, name="wo")
                        nc.tensor.matmul(wps[:],
                                         attnT_n[:, 128 * m:128 * (m + 1)],
                                         wo[:, 512 * u:512 * (u + 1)],
                                         start=True, stop=True)
                        ob = outp.tile([128, 512], F32, name="ob")
                        nc.vector.tensor_copy(ob[:], wps[:])
                        oeng = (nc.sync, nc.scalar)[(m + u) % 2] if t == 3 \
                            else nc.sync
                        oeng.dma_start(
                            out[128 * m:128 * (m + 1), 512 * u:512 * (u + 1)],
                            ob[:])

    nc.compile()
    return nc


def make_in_maps(q, k, v, Wq, bq, Wk, bk, Wv, bv, Wo, bo,
                 qk_np=np.float32, v_np=np.float32):
    f32 = np.float32
    qTa = np.ascontiguousarray(q.T).astype(qk_np)
    kTa = np.ascontiguousarray(k.T).astype(qk_np)
    vTa = np.ascontiguousarray(v.T).astype(v_np)
    WqT = np.ascontiguousarray(Wq.T)
    WkT = np.ascontiguousarray(Wk.T)
    WvT = np.ascontiguousarray(Wv.T)
    WoT = np.ascontiguousarray(Wo.T, dtype=f32)
    in_maps = []
    for c in range(NCORES):
        d0 = DL * c
        in_maps.append({
            "qT": qTa, "kT": kTa, "vT": vTa,
            "wqT": np.ascontiguousarray(WqT[:, d0:d0 + DL]).astype(qk_np),
            "wkT": np.ascontiguousarray(WkT[:, d0:d0 + DL]).astype(qk_np),
            "wvT": np.ascontiguousarray(WvT[:, d0:d0 + DL]).astype(v_np),
            "bqkv": np.ascontiguousarray(
                np.stack([bq[d0:d0 + DL], bk[d0:d0 + DL], bv[d0:d0 + DL]],
                         axis=1)).astype(f32),
            "bvrow": bv[d0:d0 + DL].reshape(1, DL).astype(v_np),
            "woT": np.ascontiguousarray(WoT[d0:d0 + DL, :]),
        })
    return in_maps


_NC_CACHE = None


def _get_nc():
    global _NC_CACHE
    if _NC_CACHE is None:
        _NC_CACHE = build_nc()
    return _NC_CACHE


def kernel(q, k, v, Wq, bq, Wk, bk, Wv, bv, Wo, bo):
    """Full-input / full-output entry point (harness contract)."""
    q, k, v = np.asarray(q), np.asarray(k), np.asarray(v)
    Wq, bq, Wk, bk = np.asarray(Wq), np.asarray(bq), np.asarray(Wk), np.asarray(bk)
    Wv, bv, Wo, bo = np.asarray(Wv), np.asarray(bv), np.asarray(Wo), np.asarray(bo)
    nc = _get_nc()
    in_maps = make_in_maps(q, k, v, Wq, bq, Wk, bk, Wv, bv, Wo, bo)
    res = run_bass_kernel_spmd(nc, in_maps, list(range(NCORES)))
    acc = res.results[0]["out"].astype(np.float64)
    for c in range(1, NCORES):
        acc += res.results[c]["out"]
    acc += bo.astype(np.float64)
    return acc.astype(np.float32)



# revision 67
# speedup vs baseline: 1.6578x; 1.6578x over previous
"""Trainium2 Bass kernel for nn_MultiHeadAttention (N=2048, D=1024, H=16, causal).

Sharding: 16 heads split across 8 NeuronCores (2 heads/core, tensor-parallel
per the sharding hint).  Each core projects Q/K/V for its 128 head-dims,
computes causal attention in scores-transposed layout ([nk, nq] blocks, exp
without max-subtraction, denominator via a ones-column in V), applies its
128-row slice of Wo, and writes a bf16 partial [2048, 1024] output.  The host
sums the 8 partials and adds bo ("all-reduce after W_o" done host-side).

v2 changes vs the 133 us baseline (which was DMA-bound: 99 us DMA busy,
83 us HWDGE dispatch for 133 descriptors-heavy DMAs):
  - all HBM traffic in bf16 (inputs 25.2->12.6 MB, outputs 8.4->4.2 MB),
    fp32 PSUM accumulation throughout; measured end-to-end rel err ~3.5e-3.
  - inputs host-prearranged to [128, 8, n] (partition-major) so each
    (tensor, nq-tile) loads with ONE big DMA: ~24 DMAs total vs 133.
  - causal trimming: diagonal-block scores/exp/PV only computed on the
    [128*i : 512] column sub-range, masking only the 128-wide triangle strip.
  - V projected directly to [n, dk] layout (no PE transposes), 4 seq-blocks
    packed into one PSUM tile.
  - softmax denominators of both heads stacked [2, 512] and broadcast with a
    single PE matmul per nq tile.
  - PSUM->SBUF output copies split across DVE and Pool(GpSimd) engines.
"""
import os
import sys

for _p in ("/opt/trn_rl_repo", "/root/.axon_site/_ro/trn_rl_repo"):
    if os.path.isdir(_p) and _p not in sys.path:
        sys.path.append(_p)

import numpy as np
import ml_dtypes

import concourse.bass as bass
import concourse.mybir as mybir
from concourse import bacc
from concourse.bass_utils import run_bass_kernel_spmd
from concourse.tile import TileContext
from contextlib import ExitStack

N = 2048
D = 1024
NCORES = 8
DL = 128

F32 = mybir.dt.float32
F32R = mybir.dt.float32r
BF16 = mybir.dt.bfloat16


def build_nc():
    nc = bacc.Bacc("TRN2", target_bir_lowering=False, debug=False,
                   num_devices=NCORES)

    # host-prearranged inputs: [p, j, n] with original row index = 128*j + p
    qp = nc.dram_tensor("qp", [128, 8, N], BF16, kind="ExternalInput")
    kp = nc.dram_tensor("kp", [128, 8, N], BF16, kind="ExternalInput")
    vp = nc.dram_tensor("vp", [128, 8, N], BF16, kind="ExternalInput")
    wqp = nc.dram_tensor("wqp", [128, 8, DL], BF16, kind="ExternalInput")
    wkp = nc.dram_tensor("wkp", [128, 8, DL], BF16, kind="ExternalInput")
    wvp = nc.dram_tensor("wvp", [128, 8, DL], BF16, kind="ExternalInput")
    bqk = nc.dram_tensor("bqk", [DL, 2], F32, kind="ExternalInput")
    wop = nc.dram_tensor("wop", [DL, D], BF16, kind="ExternalInput")
    # out[p, m, d] -> full[128*m + p, d]
    out = nc.dram_tensor("out", [128, 16, D], BF16, kind="ExternalOutput")

    AF = mybir.ActivationFunctionType

    with TileContext(nc) as tc, ExitStack() as ctx:
        const = ctx.enter_context(tc.tile_pool(name="const", bufs=1))
        big = ctx.enter_context(tc.tile_pool(name="big", bufs=1))
        xin = ctx.enter_context(tc.tile_pool(name="xin", bufs=5))
        probs_pool = ctx.enter_context(tc.tile_pool(name="probs", bufs=20))
        rc_pool = ctx.enter_context(tc.tile_pool(name="rc", bufs=3))
        nd_pool = ctx.enter_context(tc.tile_pool(name="nd", bufs=3))
        ob_pool = ctx.enter_context(tc.tile_pool(name="ob", bufs=2))
        # (probs pool holds ~20 blocks for the cross-tile PV passes)

        # ---- constants (wq first: it gates the very first matmul; the
        # rest are issued interleaved with the input loads below) ----
        wq = const.tile([128, 8, DL], BF16)
        nc.scalar.dma_start(wq[:], wqp[:])
        wk = const.tile([128, 8, DL], BF16)
        wv = const.tile([128, 8, DL], BF16)
        wo = const.tile([128, D], BF16)
        bias_qk = const.tile([128, 2], F32)
        # identity for PE transpose of the per-group attention outputs
        from concourse.masks import make_identity
        ident = const.tile([128, 128], F32)
        make_identity(nc, ident[:])

        QTs = [big.tile([128, 512], BF16, name=f"QT{t}") for t in range(4)]
        KTs = [big.tile([128, 512], BF16, name=f"KT{t}") for t in range(4)]
        Vaug0 = big.tile([128, 16, 65], BF16)
        Vaug1 = big.tile([128, 16, 65], BF16)
        nc.gpsimd.memset(Vaug0[:, :, 64:65], 1.0)
        nc.gpsimd.memset(Vaug1[:, :, 64:65], 1.0)
        attnT = big.tile([128, N], BF16)

        with tc.tile_pool(name="mm_ps", bufs=2, space="PSUM") as mm_ps, \
             tc.tile_pool(name="sc_ps", bufs=2, space="PSUM") as sc_ps, \
             tc.tile_pool(name="pv_ps", bufs=1, space="PSUM") as pv_ps:

            xq = [None] * 4
            xk = [None] * 4
            xv = [None] * 4

            def load_inputs(t):
                """One DMA per tensor for nq tile t (t=0: split for startup)."""
                xq[t] = xin.tile([128, 8, 512], BF16, name="xq")
                xk[t] = xin.tile([128, 8, 512], BF16, name="xk")
                xv[t] = xin.tile([128, 8, 512], BF16, name="xv")
                cs = slice(512 * t, 512 * (t + 1))
                if t == 0:
                    # q by j-quarters (projection consumes j in order); k and
                    # v by COLUMN halves so the first attention blocks (and
                    # the ScalarE exp stream) can start before the full tile
                    # arrives.  Weight loads are interleaved right before
                    # their first use.
                    for j in range(0, 8, 2):
                        nc.sync.dma_start(xq[t][:, j:j + 2, :],
                                          qp[:, j:j + 2, cs])
                    nc.scalar.dma_start(wk[:], wkp[:])
                    nc.scalar.dma_start(bias_qk[:], bqk[:])
                    for c in range(2):
                        nc.sync.dma_start(xk[t][:, :, 256 * c:256 * (c + 1)],
                                          kp[:, :, 256 * c:256 * (c + 1)])
                        nc.sync.dma_start(xv[t][:, :, 256 * c:256 * (c + 1)],
                                          vp[:, :, 256 * c:256 * (c + 1)])
                        if c == 0:
                            nc.scalar.dma_start(wv[:], wvp[:])
                elif t == 1:
                    for x, src in ((xq[t], qp), (xk[t], kp), (xv[t], vp)):
                        nc.sync.dma_start(x[:, 0:4, :], src[:, 0:4, cs])
                        nc.sync.dma_start(x[:, 4:8, :], src[:, 4:8, cs])
                else:
                    nc.sync.dma_start(xq[t][:], qp[:, :, cs])
                    nc.sync.dma_start(xk[t][:], kp[:, :, cs])
                    nc.sync.dma_start(xv[t][:], vp[:, :, cs])

            def proj_qk(t, k_pieces=1):
                for w, bcol, dst, xt in ((wq, 0, QTs[t], xq[t]),
                                         (wk, 1, KTs[t], xk[t])):
                    ps = mm_ps.tile([128, 512], F32, name="mm")
                    npc = k_pieces if bcol == 1 else 1
                    for c in range(npc):
                        cs2 = slice(512 // npc * c, 512 // npc * (c + 1))
                        for j in range(8):
                            nc.tensor.matmul(ps[:, cs2], w[:, j, :],
                                             xt[:, j, cs2],
                                             start=(j == 0), stop=(j == 7))
                        nc.vector.tensor_scalar_add(dst[:, cs2], ps[:, cs2],
                                                    bias_qk[:, bcol:bcol + 1])

            def proj_qk_fillers(t):
                """proj_qk(t) as 8 filler chunks of 2 matmuls each."""
                hold = {0: [None], 1: [None]}

                def chunk(w, bcol, dst, xt, c):
                    if c == 0:
                        hold[bcol][0] = mm_ps.tile([128, 512], F32,
                                                   name="mm")
                    ps = hold[bcol][0]
                    for j in (2 * c, 2 * c + 1):
                        nc.tensor.matmul(ps[:], w[:, j, :], xt[:, j, :],
                                         start=(j == 0), stop=(j == 7))
                    if c == 3:
                        nc.vector.tensor_scalar_add(
                            dst[:], ps[:], bias_qk[:, bcol:bcol + 1])

                return ([lambda c=c: chunk(wq, 0, QTs[t], xq[t], c)
                         for c in range(4)] +
                        [lambda c=c: chunk(wk, 1, KTs[t], xk[t], c)
                         for c in range(4)])

            def proj_v(t):
                # 4 seq-blocks (rows of V) packed along one PSUM tile's free
                # dim; output layout [n_within_block, dk] per block.  V bias
                # is folded into the host-side output correction (bv @ Wo.T).
                ps = mm_ps.tile([128, 512], F32, name="mm")
                for bb in range(4):
                    fs = slice(128 * bb, 128 * (bb + 1))
                    for j in range(8):
                        nc.tensor.matmul(ps[:, fs], xv[t][:, j, fs],
                                         wv[:, j, :],
                                         start=(j == 0), stop=(j == 7))
                psv = ps[:].rearrange("p (b c) -> p b c", b=4)
                nc.vector.tensor_copy(Vaug0[:, 4 * t:4 * t + 4, 0:64],
                                      psv[:, :, 0:64])
                nc.vector.tensor_copy(Vaug1[:, 4 * t:4 * t + 4, 0:64],
                                      psv[:, :, 64:128])

            def attn_tile(t, fillers=(), post_group=None, defer_last=False):
                """Causal attention for both heads, nq tile t, INCLUDING
                softmax normalization into attnT.  The block loop issues
                ONLY sc matmuls / one strided exp / one strided triangle
                mask per block (pure ScalarE streaming); probs stay in SBUF.
                PV then runs as one compact PASS per 128-row group
                (probs as stationary, 65-wide matmuls, ONE pending PSUM
                accumulation group per bank as the hardware requires), and
                each group finalizes independently: per-partition 1/denom
                normalize -> PE transpose -> attnT columns.  Passes for the
                last groups are deferred into the next tile's fillers."""
                fillers = list(fillers)
                last = 4 * t + 3
                probs_list = []
                pvg = {}
                nds = {}

                def pv_pass(g, half=None):
                    """PV pass for group g over blocks 0..4t+g; half splits
                    long passes into two filler-sized chunks."""
                    lastb = 4 * t + g
                    if g not in pvg:
                        pvg[g] = (pv_ps.tile([128, 512], F32, name="pvt0"),
                                  pv_ps.tile([128, 512], F32, name="pvt1"))
                    p0, p1 = pvg[g]
                    rng = (range(0, lastb + 1) if half is None else
                           range(0, (lastb + 1) // 2) if half == 0 else
                           range((lastb + 1) // 2, lastb + 1))
                    for b in rng:
                        pp = probs_list[b]
                        for h, pt in ((0, p0), (1, p1)):
                            Vg = (Vaug0, Vaug1)[h]
                            nc.tensor.matmul(
                                pt[:, 0:65],
                                pp[:, h, 128 * g:128 * (g + 1)],
                                Vg[:, b, 0:65],
                                start=(b == 0), stop=(b == lastb))

                def finalize_dve(g):
                    """normalize by per-partition 1/denom into [nq,d] tile."""
                    p0, p1 = pvg[g]
                    rc = rc_pool.tile([128, 2], F32, name="rc")
                    with nc.allow_low_precision(reason="plain f32 values"):
                        nc.vector.reciprocal(rc[:, 0:1], p0[:, 64:65])
                        nc.vector.reciprocal(rc[:, 1:2], p1[:, 64:65])
                    nd = nd_pool.tile([128, 128], F32, name="nd")
                    nc.vector.tensor_scalar_mul(nd[:, 0:64], p0[:, 0:64],
                                                rc[:, 0:1])
                    nc.vector.tensor_scalar_mul(nd[:, 64:128], p1[:, 0:64],
                                                rc[:, 1:2])
                    return nd

                def finalize_pe(g, nd):
                    """PE transpose of the staged group into attnT columns."""
                    m = 4 * t + g
                    tp = mm_ps.tile([128, 512], F32, name="mm")
                    nc.tensor.transpose(tp[:, 0:128], nd[:], ident[:])
                    nc.vector.tensor_copy(
                        attnT[:, 128 * m:128 * (m + 1)], tp[:, 0:128])
                    if post_group is not None:
                        post_group(g)

                for b in range(last + 1):
                    off = 128 * (b - 4 * t) if b >= 4 * t else 0
                    kslc = slice(128 * (b % 4), 128 * (b % 4 + 1))
                    sc = sc_ps.tile([128, 2, 512], F32, name="sc")
                    for h in range(2):
                        hs = slice(64 * h, 64 * (h + 1))
                        nc.tensor.matmul(
                            sc[:, h, off:512], KTs[b // 4][hs, kslc],
                            QTs[t][hs, off:512],
                            start=True, stop=True, tile_position=(64 * h, 0))
                    probs = probs_pool.tile([128, 2, 512], BF16, name="probs")
                    nc.scalar.activation(probs[:, :, off:512],
                                         sc[:, :, off:512], AF.Exp,
                                         scale=0.125)
                    if b >= 4 * t:
                        nc.gpsimd.affine_select(
                            out=probs[:, :, off:off + 128],
                            in_=probs[:, :, off:off + 128],
                            compare_op=mybir.AluOpType.is_ge, fill=0.0,
                            base=0, pattern=[[0, 2], [1, 128]],
                            channel_multiplier=-1)
                    probs_list.append(probs)
                    if b < last:
                        blocks_left = last - b
                        pops = min(len(fillers), 2 if
                                   len(fillers) > blocks_left else 1)
                        for _ in range(pops):
                            fillers.pop(0)()
                    # in-loop PV passes: group g once its probs exist
                    g = b - 4 * t - 1
                    if g >= 0:
                        pv_pass(g)
                        if g >= 1:
                            finalize_pe(g - 1, nds[g - 1])
                        nds[g] = finalize_dve(g)
                for f in fillers:
                    f()
                nds_state = nds

                def d_pass3a():
                    pv_pass(3, half=0)

                def d_pass3b():
                    pv_pass(3, half=1)

                def d_fin2pe():
                    finalize_pe(2, nds_state[2])

                def d_fin3dve():
                    nds_state[3] = finalize_dve(3)

                def d_fin3pe():
                    finalize_pe(3, nds_state[3])

                deferred_work = [d_fin2pe, d_pass3a, d_pass3b, d_fin3dve,
                                 d_fin3pe]
                if defer_last:
                    return deferred_work
                for f in deferred_work:
                    f()
                return []

            obs = [None] * 4

            def wo_piece(t, i, u, eng=None):
                """One [128,512] piece of the Wo projection for row-block
                4t+i, half u; DMA fires when the tile's 8 pieces are done
                (t=3: per-row-block DMAs to shorten the kernel tail)."""
                if i == 0 and u == 0:
                    obs[t] = ob_pool.tile([128, 4, D], BF16, name="ob")
                if t == 3 and (2 * i + u) % 2 == 1:
                    # sc pool is idle after the last exp; alternating pools
                    # doubles the PSUM rotation depth for the tail pieces
                    wps = sc_ps.tile([128, 2, 512], F32, name="sc")[:, 0, :]
                else:
                    wps = mm_ps.tile([128, 512], F32, name="mm")[:]
                nc.tensor.matmul(wps,
                                 attnT[:, 128 * (4 * t + i):
                                       128 * (4 * t + i + 1)],
                                 wo[:, 512 * u:512 * (u + 1)],
                                 start=True, stop=True)
                if eng is None:
                    eng = nc.vector
                dst = obs[t][:, i, 512 * u:512 * (u + 1)]
                if eng is nc.scalar:
                    eng.copy(dst, wps)
                else:
                    eng.tensor_copy(dst, wps)
                if t == 3 and u == 1:
                    nc.scalar.dma_start(out[:, 12 + i, :], obs[t][:, i, :])
                elif i == 3 and u == 1:
                    nc.scalar.dma_start(out[:, 4 * t:4 * t + 4, :],
                                        obs[t][:])

            def v_block_filler(t):
                """proj_v(t) one seq-block at a time, usable as attention
                fillers: block bb lands just before its PV consumes it."""
                vps = [None]

                def fill(bb):
                    if bb == 0:
                        vps[0] = mm_ps.tile([128, 512], F32, name="mm")
                    ps = vps[0]
                    fs = slice(128 * bb, 128 * (bb + 1))
                    for j in range(8):
                        nc.tensor.matmul(ps[:, fs], xv[t][:, j, fs],
                                         wv[:, j, :],
                                         start=(j == 0), stop=(j == 7))
                    nc.vector.tensor_copy(
                        Vaug0[:, 4 * t + bb, 0:64],
                        ps[:, 128 * bb:128 * bb + 64])
                    nc.vector.tensor_copy(
                        Vaug1[:, 4 * t + bb, 0:64],
                        ps[:, 128 * bb + 64:128 * (bb + 1)])
                return [lambda bb=bb: fill(bb) for bb in range(4)]

            def wo_fill(t):
                return [(lambda i=i, u=u: wo_piece(t, i, u))
                        for i in range(4) for u in range(2)]

            # ---- software pipeline over the 4 nq tiles ----
            # Fillers enter the PE stream inside the (ScalarE-paced)
            # attention block loops:
            #   attn(0) <- V(0) blocks      attn(1) <- Wo(0) pieces
            #   attn(2) <- Wo(1) pieces     attn(3) <- V(3) blocks + Wo(2)
            load_inputs(0)
            load_inputs(1)
            nc.scalar.dma_start(wo[:], wop[:])
            proj_qk(0, k_pieces=2)
            deferred = []
            for t in range(4):
                wo_prev = [(lambda i=i, u=u: wo_piece(t - 1, i, u))
                           for i in range(4) for u in range(2)]
                # next tile's Q/K projection runs INSIDE this tile's block
                # loop so the ScalarE exp stream never drains at a tile
                # boundary; Wo of the previous tile follows.
                fillers = (v_block_filler(0) if t == 0 else
                           proj_qk_fillers(2) + wo_prev if t == 1 else
                           proj_qk_fillers(3) + wo_prev if t == 2 else
                           v_block_filler(3) + wo_prev)
                fillers = deferred + fillers
                post = None
                if t == 3:
                    # last tile: project+store each row block the moment its
                    # attnT columns land, so almost nothing remains after
                    # the final attention block
                    def post(g):
                        wo_piece(3, g, 0, eng=nc.vector)
                        wo_piece(3, g, 1, eng=nc.scalar)
                deferred = attn_tile(t, fillers, post_group=post,
                                     defer_last=(t < 3))
                if t < 2:
                    load_inputs(t + 2)
                if t == 0:
                    proj_qk(1)
                if t < 2:
                    proj_v(t + 1)

    nc.compile()
    return nc


def make_in_maps(q, k, v, Wq, bq, Wk, bk, Wv, bv, Wo, bo):
    bf = ml_dtypes.bfloat16

    def arrange(xT):
        # [1024, cols] -> [128, 8, cols] with row = 128*j + p
        return np.ascontiguousarray(
            xT.reshape(8, 128, -1).swapaxes(0, 1)).astype(bf)

    qp = arrange(np.ascontiguousarray(q.T))
    kp = arrange(np.ascontiguousarray(k.T))
    vp = arrange(np.ascontiguousarray(v.T))
    WqT, WkT, WvT = Wq.T, Wk.T, Wv.T
    WoT = np.ascontiguousarray(Wo.T)
    in_maps = []
    for c in range(NCORES):
        d0 = DL * c
        in_maps.append({
            "qp": qp, "kp": kp, "vp": vp,
            "wqp": arrange(np.ascontiguousarray(WqT[:, d0:d0 + DL])),
            "wkp": arrange(np.ascontiguousarray(WkT[:, d0:d0 + DL])),
            "wvp": arrange(np.ascontiguousarray(WvT[:, d0:d0 + DL])),
            "bqk": np.ascontiguousarray(
                np.stack([bq[d0:d0 + DL], bk[d0:d0 + DL]],
                         axis=1)).astype(np.float32),
            "wop": np.ascontiguousarray(WoT[d0:d0 + DL, :]).astype(bf),
        })
    return in_maps


_NC_CACHE = None


def _get_nc():
    global _NC_CACHE
    if _NC_CACHE is None:
        _NC_CACHE = build_nc()
    return _NC_CACHE


def kernel(q, k, v, Wq, bq, Wk, bk, Wv, bv, Wo, bo):
    """Full-input / full-output entry point (harness contract)."""
    q, k, v = np.asarray(q), np.asarray(k), np.asarray(v)
    Wq, bq, Wk, bk = np.asarray(Wq), np.asarray(bq), np.asarray(Wk), np.asarray(bk)
    Wv, bv, Wo, bo = np.asarray(Wv), np.asarray(bv), np.asarray(Wo), np.asarray(bo)
    nc = _get_nc()
    in_maps = make_in_maps(q, k, v, Wq, bq, Wk, bk, Wv, bv, Wo, bo)
    res = run_bass_kernel_spmd(nc, in_maps, list(range(NCORES)))
    acc = np.zeros((N, D), np.float64)
    for c in range(NCORES):
        # out[p, m, d] -> rows 128*m + p
        part = np.asarray(res.results[c]["out"]).astype(np.float64)
        acc += part.swapaxes(0, 1).reshape(N, D)
    # V-bias term folded out of the device kernel: P @ (V + bv) @ Wo.T
    # = P @ V @ Wo.T + bv @ Wo.T (softmax rows sum to 1), plus bo.
    acc += (bv.astype(np.float64) @ Wo.T.astype(np.float64)
            + bo.astype(np.float64))
    return acc.astype(np.float32)
